# revision 7
# baseline (speedup 1.0000x reference)
"""ContactAwareLoss Trainium2 kernel.

Strategy: pure data-parallel over batch (512 rows -> 8 cores x 64 rows).
Each core computes four partial sums over its shard:
  [0] sum_{t,h} probs2 * |dist - 0.1|            (contact distance, unnormalized)
  [1] sum_{j,h} probs2[j+1] * ||r[j+1]-r[j]||     (contact velocity, unnormalized)
  [2] 2 * sum first_contact * (5-tap sum of |second diff of dist|)
  [3] 2 * sum first_contact                       (count; x2 from the +-1 Sign encoding)
The host divides by the global element counts / count and applies the ramp.

On-chip layout: partition p = half*64 + b (sequence halved so 64 batch rows
fill 128 partitions).  Inputs live in RESIDENT padded tiles [P, HS+6, ...]
loaded once per run: resident col x <-> local time t = x-3.  The 3-col pads
hold the half-boundary neighbours (real data via small DMAs) or zeros at the
global sequence ends; the zero probs pad also masks the j=seq-1 velocity
product for free.  Compute then runs in time chunks (narrow first chunk so
the pipeline fill is short), reading offset views of the residents.

DMA: residents are filled by a few column-split dma_starts per tensor/half
with growing split sizes -- descriptors are one full row-range per batch row
(4-16.5KB), which runs near SDMA line rate, while the small first split lets
chunk-0 compute start after ~2us.  The outermost AP dim is bs_local (64) so
descriptors spread over all 16 SDMA engines (outer-dim -> engine assignment).

Engine split (measured modes):
 - DVE: all tensor_tensor work in bf16 contiguous (2x packed mode); the
   geometry runs in a (c, t, h) layout so channel sums are contiguous adds;
   r is ONE fused fp32 subtract that deinterleaves hand and broadcasts obj
   via a stride-0 AP.  fc = max(q,0) uses tensor_scalar WITHOUT accum_out
   (accum forces 1x; no-accum runs 4x).
 - ACT: Square/Sqrt/Abs chains + Sign for the contact threshold (strided
   fp32 inputs run at full ACT speed; exact fp32 compare, +-1 encoding).
 - TensorE (idle otherwise): all four row-sum reductions as ones-vector
   matmuls accumulating into PSUM across chunks (host only needs global
   sums).  GpSimd: unused (measured ~10x below its cost model).
Statement order = per-engine issue order, interleaved so each engine has
ready work while the other runs long ops.
"""

import numpy as np

BS, SEQ = 512, 4096
N_CORES = 8
CHUNKS = (128, 512, 512, 512, 384)   # compute chunk widths; sum = HS
SPLITS = (160, 512, 688, 688)        # DMA column splits; sum = HS


def build_nc(bs_local, seq, chunks, splits):
    import concourse.bass as bass
    import concourse.bacc as bacc
    import concourse.tile as tile
    from concourse import mybir

    f32 = mybir.dt.float32
    bf16 = mybir.dt.bfloat16
    Alu = mybir.AluOpType
    Act = mybir.ActivationFunctionType

    P = 2 * bs_local          # partitions used
    HS = seq // 2             # timesteps per partition row
    assert sum(chunks) == HS and sum(splits) == HS
    C = len(chunks)
    H = P // 2
    R = HS + 6                # resident tile width (3-col pads both sides)

    nc = bacc.Bacc("TRN2", target_bir_lowering=False, debug=False)
    hand = nc.dram_tensor("pred_hand_pos", [bs_local, seq, 2, 3], f32, kind="ExternalInput")
    obj = nc.dram_tensor("pred_obj_pos", [bs_local, seq, 3], f32, kind="ExternalInput")
    probs = nc.dram_tensor("contact_probs", [bs_local, seq, 3], f32, kind="ExternalInput")
    partials = nc.dram_tensor("partials", [P, 4], f32, kind="ExternalOutput")

    def dram_ap(t, offset, dims):
        return bass.AP(tensor=t, offset=offset, ap=[list(d) for d in dims])

    def fview(t, off, dims):
        """Free-dim view of a tile at element offset `off`."""
        return bass.AP(tensor=t.tensor, offset=t[:].offset + off,
                       ap=[t[:].ap[0]] + [list(d) for d in dims])

    with tile.TileContext(nc) as tc:
        import contextlib
        with contextlib.ExitStack() as ctx:
            inp = ctx.enter_context(tc.tile_pool(name="inp", bufs=1))
            work = ctx.enter_context(tc.tile_pool(name="work", bufs=2))
            singles = ctx.enter_context(tc.tile_pool(name="singles", bufs=1))
            psum = ctx.enter_context(tc.tile_pool(name="psum", bufs=1, space="PSUM"))

            outt = singles.tile([P, 4], f32)
            nc.vector.memset(outt[:], 0.0)
            c_neg01 = singles.tile([P, 1], f32)
            nc.vector.memset(c_neg01[:], -0.1)
            c_neg05 = singles.tile([P, 1], f32)
            nc.vector.memset(c_neg05[:], -0.5)
            ones = singles.tile([P, 1], bf16)
            nc.vector.memset(ones[:], 1.0)

            accs = [psum.tile([1, 512], f32, name=f"acc{i}", tag=f"acc{i}")
                    for i in range(4)]

            # ---- resident input tiles ----
            hand_r = inp.tile([P, R, 2, 3], f32)
            obj_r = inp.tile([P, R, 3], f32)
            probs_r = inp.tile([P, R, 3], f32)
            tensors = (
                (hand_r, hand, 6, nc.sync),
                (obj_r, obj, 3, nc.scalar),
                (probs_r, probs, 3, nc.scalar),
            )
            # zero pads (global sequence ends)
            for tile_buf, _, _, _ in tensors:
                nc.vector.memset(tile_buf[0:H, 0:3], 0.0)
                nc.vector.memset(tile_buf[H:P, HS + 3:R], 0.0)
            # split loads, interleaved across tensors so early columns of all
            # three tensors land first
            s0 = 0
            for si, sw in enumerate(splits):
                for tile_buf, ten, k, eng in tensors:
                    for h in range(2):
                        eng.dma_start(
                            out=tile_buf[h * H:(h + 1) * H, 3 + s0:3 + s0 + sw],
                            in_=dram_ap(ten, (h * HS + s0) * k,
                                        [[seq * k, bs_local], [1, sw * k]]),
                        )
                    if si == 0:  # half-boundary neighbours: half-1 left pad
                        eng.dma_start(
                            out=tile_buf[H:P, 0:3],
                            in_=dram_ap(ten, (HS - 3) * k,
                                        [[seq * k, bs_local], [1, 3 * k]]),
                        )
                    if si == len(splits) - 1:  # half-0 right pad
                        eng.dma_start(
                            out=tile_buf[0:H, HS + 3:HS + 6],
                            in_=dram_ap(ten, HS * k,
                                        [[seq * k, bs_local], [1, 3 * k]]),
                        )
                s0 += sw

            t0 = 0
            for c, W in enumerate(chunks):
                E = W + 6

                # ACT: cb first (needs only probs; exact fp32 threshold, +-1)
                cb_t = work.tile([P, W + 1, 2], bf16, tag="cb")
                nc.scalar.activation(cb_t[:], probs_r[:, t0 + 2:t0 + 3 + W, 0:2],
                                     Act.Sign, bias=c_neg05[:])

                # DVE: probsb cast + fused r while ACT runs cb
                probsb_t = work.tile([P, W + 1, 2], bf16, tag="probsb")
                nc.vector.tensor_copy(probsb_t[:], probs_r[:, t0 + 3:t0 + 4 + W, 0:2])
                # r[c,t,h] = hand[t,h,c] - obj[t,c]  (fp32 -> bf16, deinterleave
                # + stride-0 obj broadcast in one op)
                r_t = work.tile([P, 3, E, 2], bf16, tag="r")
                hand_v = fview(hand_r, t0 * 6, [[1, 3], [6, E], [3, 2]])
                obj_v = fview(obj_r, t0 * 3, [[1, 3], [3, E], [0, 2]])
                nc.vector.tensor_sub(r_t[:], hand_v, obj_v)

                # ACT: sq = r^2 while DVE does dr/q/fc
                sq_t = work.tile([P, 3, E, 2], bf16, tag="r")
                nc.scalar.activation(sq_t[:], r_t[:], Act.Square)

                dr_t = work.tile([P, 3, W, 2], bf16, tag="dr")
                nc.vector.tensor_sub(dr_t[:], r_t[:, :, 4:4 + W, :], r_t[:, :, 3:3 + W, :])
                q_t = work.tile([P, W, 2], bf16, tag="q")
                nc.vector.tensor_sub(q_t[:], cb_t[:, 1:W + 1, :], cb_t[:, 0:W, :])
                if c == 0:
                    nc.vector.memset(q_t[0:H, 0:3, :], 0.0)  # t<3 (incl. forced-false t=0)
                if c == C - 1:
                    nc.vector.memset(q_t[H:P, W - 3:W, :], 0.0)  # t >= seq-3
                fc_t = work.tile([P, W, 2], bf16, tag="fc")
                nc.vector.tensor_scalar(
                    out=fc_t[:], in0=q_t[:], scalar1=0.0, scalar2=None, op0=Alu.max)

                # ACT: dsq = dr^2 (dr just finished)
                dsq_t = work.tile([P, 3, W, 2], bf16, tag="dr")
                nc.scalar.activation(dsq_t[:], dr_t[:], Act.Square)

                # DVE: d2 channel sums (sq ready by now)
                d2a_t = work.tile([P, E, 2], bf16, tag="tmpa")
                nc.vector.tensor_add(d2a_t[:], sq_t[:, 0], sq_t[:, 1])
                d2_t = work.tile([P, E, 2], bf16, tag="d2")
                nc.vector.tensor_add(d2_t[:], d2a_t[:], sq_t[:, 2])

                # ACT: d = sqrt(d2)
                d_t = work.tile([P, E, 2], bf16, tag="d")
                nc.scalar.activation(d_t[:], d2_t[:], Act.Sqrt)

                # DVE: v2 channel sums (dsq ready)
                v2a_t = work.tile([P, W, 2], bf16, tag="tmpa")
                nc.vector.tensor_add(v2a_t[:], dsq_t[:, 0], dsq_t[:, 1])
                v2_t = work.tile([P, W, 2], bf16, tag="v2")
                nc.vector.tensor_add(v2_t[:], v2a_t[:], dsq_t[:, 2])

                # ACT: vd = sqrt(v2); derr = |d - 0.1|
                vd_t = work.tile([P, W, 2], bf16, tag="vd")
                nc.scalar.activation(vd_t[:], v2_t[:], Act.Sqrt)
                # (no vd edge mask needed: the zero probs pad kills the
                #  j=seq-1 product below)
                derr_t = work.tile([P, W, 2], bf16, tag="derr")
                nc.scalar.activation(derr_t[:], d_t[:, 3:3 + W, :], Act.Abs, bias=c_neg01[:])

                # DVE: smoothness first diffs (d ready)
                e_t = work.tile([P, E - 1, 2], bf16, tag="e")
                nc.vector.tensor_sub(e_t[:], d_t[:, 1:E, :], d_t[:, 0:E - 1, :])
                sdp_t = work.tile([P, W + 4, 2], bf16, tag="sdp")
                nc.vector.tensor_sub(sdp_t[:], e_t[:, 0:W + 4, :], e_t[:, 1:W + 5, :])

                # ACT: sd = |sdp|
                sd_t = work.tile([P, W + 4, 2], bf16, tag="sd")
                nc.scalar.activation(sd_t[:], sdp_t[:], Act.Abs)

                # DVE: weight products + 5-tap movsum
                z2_t = work.tile([P, W, 2], bf16, tag="z2")
                nc.vector.tensor_mul(z2_t[:], probsb_t[:, 1:W + 1, :], vd_t[:])
                z1_t = work.tile([P, W, 2], bf16, tag="z1")
                nc.vector.tensor_mul(z1_t[:], probsb_t[:, 0:W, :], derr_t[:])
                s2_t = work.tile([P, W + 3, 2], bf16, tag="s2")
                nc.vector.tensor_add(s2_t[:], sd_t[:, 0:W + 3, :], sd_t[:, 1:W + 4, :])
                s4_t = work.tile([P, W + 1, 2], bf16, tag="s4")
                nc.vector.tensor_add(s4_t[:], s2_t[:, 0:W + 1, :], s2_t[:, 2:W + 3, :])
                sm5_t = work.tile([P, W, 2], bf16, tag="sm5")
                nc.vector.tensor_add(sm5_t[:], s4_t[:, 0:W, :], sd_t[:, 4:W + 4, :])
                z3_t = work.tile([P, W, 2], bf16, tag="z3")
                nc.vector.tensor_mul(z3_t[:], sm5_t[:], fc_t[:])

                # ---- reductions on TensorE: ones^T @ z -> PSUM column sums ----
                for acc, zt in zip(accs, (z1_t, z2_t, z3_t, fc_t)):
                    zv = fview(zt, 0, [[1, 2 * W]])
                    for off in range(0, 2 * W, 512):
                        n = min(512, 2 * W - off)
                        nc.tensor.matmul(
                            out=acc[0:1, 0:n], lhsT=ones[:], rhs=zv[:, off:off + n],
                            start=(c == 0 and off == 0), stop=(c == C - 1 and off + n == 2 * W),
                            skip_group_check=True)

                t0 += W

            # ---- final: reduce each PSUM accumulator row to one scalar ----
            for i, acc in enumerate(accs):
                nc.vector.tensor_reduce(outt[0:1, i:i + 1], acc[:], axis=mybir.AxisListType.X, op=Alu.add)
            nc.sync.dma_start(out=partials.ap(), in_=outt[:])

    nc.compile()
    return nc


_cache = {}


def _get_nc(bs_local, seq, chunks, splits):
    key = (bs_local, seq, chunks, splits)
    if key not in _cache:
        _cache[key] = build_nc(bs_local, seq, chunks, splits)
    return _cache[key]


def combine_partials(parts, bs, seq, training_step):
    """parts: float array [..., 4] of per-core/per-partition partial sums.

    Slots [2] (sm total) and [3] (count) are doubled by the +-1 Sign encoding.
    """
    s = np.asarray(parts, dtype=np.float64).reshape(-1, 4).sum(axis=0)
    l1 = s[0] / (bs * seq * 2)
    l2 = s[1] / (bs * (seq - 1) * 2) if seq > 1 else 0.0
    cnt = s[3] / 2.0
    sm = (s[2] / 2.0 / 5.0) / max(cnt, 1.0) if (seq > 5 and cnt > 0) else 0.0
    ramp = min(1.0, float(training_step) / 1000.0)
    return np.array(ramp * (1.0 * l1 + 0.5 * l2 + 0.3 * sm), dtype=np.float32)


def _run(pred_hand_pos, pred_obj_pos, contact_probs, **spmd_kwargs):
    from concourse.bass_utils import run_bass_kernel_spmd

    hand = np.ascontiguousarray(np.asarray(pred_hand_pos, dtype=np.float32))
    obj = np.ascontiguousarray(np.asarray(pred_obj_pos, dtype=np.float32))
    probs = np.ascontiguousarray(np.asarray(contact_probs, dtype=np.float32))
    bs, seq = hand.shape[:2]
    bs_local = bs // N_CORES
    nc = _get_nc(bs_local, seq, CHUNKS, SPLITS)

    in_maps = []
    for i in range(N_CORES):
        sl = slice(i * bs_local, (i + 1) * bs_local)
        in_maps.append({
            "pred_hand_pos": hand[sl],
            "pred_obj_pos": obj[sl],
            "contact_probs": probs[sl],
        })
    # The axon terminal occasionally reports the exec unit unrecoverable on
    # the first touch after a previous process's teardown; a retry lands on a
    # recovered device.
    last_err = None
    for _ in range(3):
        try:
            res = run_bass_kernel_spmd(
                nc, in_maps, core_ids=list(range(N_CORES)), **spmd_kwargs
            )
            parts = np.stack([res.results[i]["partials"] for i in range(N_CORES)])
            return parts, res
        except Exception as e:  # noqa: BLE001
            last_err = e
    raise last_err


def kernel(pred_hand_pos, pred_obj_pos, contact_probs, training_step):
    bs, seq = np.asarray(pred_hand_pos).shape[:2]
    parts, _ = _run(pred_hand_pos, pred_obj_pos, contact_probs)
    return combine_partials(parts, bs, seq, training_step)
